# revision 2
# baseline (speedup 1.0000x reference)
"""Trainium2 Bass kernel for a top-2 gated MoE layer (8 experts, H=1024, F=4096).

Strategy (expert parallelism across the 8 NeuronCores):
  - Host computes the top-2 routing, the combine weights
    comb = softmax(top2) * alpha, and applies LayerNorm to x in fp32;
    it gathers each expert's tokens into a padded capacity-C block and
    pre-arranges EVERY device transfer as its own contiguous DRAM slab,
    already in SBUF [partition, k, col] layout.
  - Each core runs one expert (matmuls in bf16, fp32 PSUM accumulation).
  - Chunk widths are all >= ~233 columns so each LDWEIGHTS (~97 ns) hides
    under its matmuls; the first chunk is small (256) so compute starts
    as early as the DMA stream allows.
  - Startup-critical transfers (x chunk 0, w1 pieces) go first on both
    hardware DGE rings; small latency-bound tensors (b1, b2, comb) are
    single-packet transfers placed off the critical path.
  - Host scatter-adds the per-expert outputs back into the full [B,S,H].

Self-contained: shapes are hardcoded from the problem spec.
"""

import numpy as np
import ml_dtypes
from contextlib import ExitStack

TOP_K = 2
LN_EPS = 1e-5
B, S, H, E, F = 2, 2048, 1024, 8, 4096
T = B * S
P = 128
KH = H // P          # 8 H-tiles
FB = 1024            # F block size
NFB = F // FB        # 4 blocks
MF = FB // P         # 8 F-tiles per block
NQ = 4               # block-0 w2 quarters
NW1 = 8              # block-0 w1 pieces (one per F-tile)

_BUILD_CACHE = {}


def _chunks(C):
    # First chunk small (256) for an early compute start; middle chunks 512;
    # tail split so every chunk is in [234, 512] (LDWEIGHTS stays hidden).
    out = [(0, min(256, C))]
    off = out[0][1]
    rem = C - off
    while rem > 1024:
        out.append((off, 512))
        off += 512
        rem -= 512
    if rem > 512:
        a = min(512, rem - 234)
        out.append((off, a))
        off += a
        rem -= a
    if rem:
        out.append((off, rem))
    return out


def _build(C):
    """Build + compile the single-core Bass program (SPMD across 8 cores)."""
    if C in _BUILD_CACHE:
        return _BUILD_CACHE[C]

    import concourse.bass as bass  # noqa: F401
    import concourse.tile as tile
    import concourse.mybir as mybir
    from concourse import bacc

    bf = mybir.dt.bfloat16
    f32 = mybir.dt.float32
    AF = mybir.ActivationFunctionType
    OP = mybir.AluOpType

    nc = bacc.Bacc("TRN2", target_bir_lowering=False, debug=False, num_devices=8)

    chunks = _chunks(C)
    NC = len(chunks)
    d_x = [nc.dram_tensor(f"x{ci}", [P, KH, w], bf, kind="ExternalInput")
           for ci, (off, w) in enumerate(chunks)]
    d_w1q = [nc.dram_tensor(f"w1q{q}", [P, KH, FB // NW1], bf,
                            kind="ExternalInput") for q in range(NW1)]
    d_w2q = [nc.dram_tensor(f"w2q{q}", [P, MF, H // NQ], bf,
                            kind="ExternalInput") for q in range(NQ)]
    d_w1b = [nc.dram_tensor(f"w1b{fb}", [P, KH, FB], bf,
                            kind="ExternalInput") for fb in range(1, NFB)]
    d_w2b = [nc.dram_tensor(f"w2b{fb}", [P, MF, H], bf,
                            kind="ExternalInput") for fb in range(1, NFB)]
    d_b1r = nc.dram_tensor("b1r", [P, F // P], f32, kind="ExternalInput")
    d_b2 = nc.dram_tensor("b2", [P, KH], f32, kind="ExternalInput")
    d_comb = nc.dram_tensor("combr", [P, C], bf, kind="ExternalInput")
    d_y = [nc.dram_tensor(f"y{ci}", [P, KH, w], bf, kind="ExternalOutput")
           for ci, (off, w) in enumerate(chunks)]

    with tile.TileContext(nc) as tc, ExitStack() as ctx:
        const = ctx.enter_context(tc.tile_pool(name="const", bufs=1))
        bpool = ctx.enter_context(tc.tile_pool(name="bcast", bufs=1))
        xpool = ctx.enter_context(tc.tile_pool(name="x", bufs=1))
        w1a = ctx.enter_context(tc.tile_pool(name="w1a", bufs=NW1))
        w2a = ctx.enter_context(tc.tile_pool(name="w2a", bufs=NQ))
        w1pool = ctx.enter_context(tc.tile_pool(name="w1", bufs=2))
        w2pool = ctx.enter_context(tc.tile_pool(name="w2", bufs=2))
        apool = ctx.enter_context(tc.tile_pool(name="acts", bufs=8))
        ypool = ctx.enter_context(tc.tile_pool(name="yacc", bufs=1))
        ps_small = ctx.enter_context(
            tc.tile_pool(name="ps_small", bufs=1, space="PSUM"))
        ps1 = ctx.enter_context(tc.tile_pool(name="ps1", bufs=4, space="PSUM"))
        ps2 = ctx.enter_context(tc.tile_pool(name="ps2", bufs=3, space="PSUM"))

        # ---- PE warm-up: junk matmuls train the HAM clock gate toward
        # 2.4 GHz and bridge until the first x/w DMAs land ----
        ones_k = const.tile([P, 1], bf)
        nc.vector.memset(ones_k, 1.0)
        warm_rhs = const.tile([P, 512], bf)
        nc.vector.memset(warm_rhs, 0.0)
        ps_w = ps_small.tile([1, 512], f32, tag="pss", name="warm")
        for i in range(12):
            nc.tensor.matmul(ps_w[:], ones_k[:], warm_rhs[:],
                             start=True, stop=True)
        for i in range(4):
            nc.tensor.matmul(ps_w[:, 0:128], ones_k[:], warm_rhs[:, 0:128],
                             start=True, stop=True)

        xt = [xpool.tile([P, KH, w], bf, tag=f"xc{ci}", name=f"x_{ci}")
              for ci, (off, w) in enumerate(chunks)]
        w1q = [w1a.tile([P, KH, FB // NW1], bf, tag="w1a", name=f"w1a_{q}")
               for q in range(NW1)]
        w2q = [w2a.tile([P, MF, H // NQ], bf, tag="w2a", name=f"w2a_{q}")
               for q in range(NQ)]
        b1_sb = const.tile([P, F // P], f32)
        b2_sb = const.tile([P, KH], f32)
        comb_b = bpool.tile([P, C], bf)

        def xk(ci, k):
            return xt[ci][:, k, :]

        # ---- DMA: startup-critical slabs first on BOTH hwdge rings ----
        # sync ring:   x0(k0-3), w1q even pieces, x1(k0-3), w2q even, ...
        # scalar ring: x0(k4-7), w1q odd pieces, b1, x1(k4-7), x2.., w2q odd
        nc.sync.dma_start(xt[0][:, 0:KH // 2, :],
                          d_x[0].ap()[:, 0:KH // 2, :])
        nc.scalar.dma_start(xt[0][:, KH // 2:KH, :],
                            d_x[0].ap()[:, KH // 2:KH, :])
        for q in range(NW1):
            eng = nc.sync if q % 2 == 0 else nc.scalar
            eng.dma_start(w1q[q][:], d_w1q[q].ap())
        # b1 feeds the fc1 fixups (first needed ~20us in); single packet
        nc.scalar.dma_start(b1_sb[:], d_b1r.ap(), single_packet=True)
        if NC > 1:
            nc.sync.dma_start(xt[1][:, 0:KH // 2, :],
                              d_x[1].ap()[:, 0:KH // 2, :])
            nc.scalar.dma_start(xt[1][:, KH // 2:KH, :],
                                d_x[1].ap()[:, KH // 2:KH, :])
        for ci in range(2, NC):
            nc.scalar.dma_start(xt[ci][:], d_x[ci].ap())
        for q in range(NQ):
            eng = nc.sync if q % 2 == 0 else nc.scalar
            eng.dma_start(w2q[q][:], d_w2q[q].ap())
        # finalize-only tensors, well off the critical path
        nc.scalar.dma_start(b2_sb[:], d_b2.ap(), single_packet=True)
        nc.scalar.dma_start(comb_b[:], d_comb.ap())

        ybig = ypool.tile([P, KH, C], bf, tag="yacc", name="ybig")
        y_acc = [ybig[:, h, :] for h in range(KH)]

        def load_w_block(fb):
            w1blk = w1pool.tile([P, KH, FB], bf, tag="w1", name=f"w1_{fb}")
            nc.sync.dma_start(w1blk[:], d_w1b[fb - 1].ap())
            w2blk = w2pool.tile([P, MF, H], bf, tag="w2", name=f"w2_{fb}")
            nc.scalar.dma_start(w2blk[:], d_w2b[fb - 1].ap())
            return ([w1blk[:, k, :] for k in range(KH)],
                    [w2blk[:, k, :] for k in range(MF)])

        at0 = [apool.tile([P, C], bf, tag="acts", name=f"a_0_{m}")
               for m in range(MF)]

        def fc1_fixup(psum, dst, fcol):
            # x is fully LayerNormed on host; just bias + gelu
            nc.scalar.activation(dst, psum[:], AF.Gelu_apprx_tanh,
                                 bias=b1_sb[:, fcol:fcol + 1])

        # ---- F-block 0: all fc1 chunk by chunk (earliest start on the
        # smallest chunk), then all fc2 (w2q needed only once fc1 is done) --
        for ci in range(NC):
            off, w = chunks[ci]
            sl = slice(off, off + w)
            for m in range(MF):
                pst = ps1.tile([P, w], f32, tag="ps1", name=f"ps1_0_{m}_{ci}")
                for k in range(KH):
                    nc.tensor.matmul(pst[:], w1q[m][:, k, :],
                                     xk(ci, k),
                                     start=(k == 0), stop=(k == KH - 1))
                fc1_fixup(pst, at0[m][:, sl], m)
        for ci in range(NC):
            off, w = chunks[ci]
            sl = slice(off, off + w)
            for h in range(KH):
                w2piece = w2q[h // 2]
                hsl = slice((h % 2) * P, (h % 2) * P + P)
                pst = ps2.tile([P, w], f32, tag="ps2", name=f"ps2_0_{h}_{ci}")
                for k in range(MF):
                    nc.tensor.matmul(pst[:], w2piece[:, k, hsl],
                                     at0[k][:, sl],
                                     start=(k == 0), stop=(k == MF - 1))
                nc.scalar.activation(y_acc[h][:, sl], pst[:], AF.Identity,
                                     bias=0.0)

        # ---- remaining F blocks: weight-stationary (each lhsT feeds all
        # chunks); the last block splits off the final chunk alone so the
        # finalize tail is short. ----
        for fb in range(1, NFB):
            w1t, w2t = load_w_block(fb)

            at = [apool.tile([P, C], bf, tag="acts", name=f"a_{fb}_{m}")
                  for m in range(MF)]
            if fb == NFB - 1 and NC > 1:
                groups = [list(range(NC - 1)), [NC - 1]]
            else:
                groups = [list(range(NC))]

            for cig in groups:
                for m in range(MF):
                    psg = {ci: ps1.tile([P, chunks[ci][1]], f32, tag="ps1",
                                        name=f"ps1_{fb}_{m}_{ci}")
                           for ci in cig}
                    for k in range(KH):
                        lhsT = w1t[k][:, m * P:(m + 1) * P]
                        for ci in cig:
                            nc.tensor.matmul(psg[ci][:], lhsT, xk(ci, k),
                                             start=(k == 0), stop=(k == KH - 1))
                    fcol = fb * MF + m
                    for ci in cig:
                        off, w = chunks[ci]
                        fc1_fixup(psg[ci], at[m][:, off:off + w], fcol)
                for h in range(KH):
                    psg = {ci: ps2.tile([P, chunks[ci][1]], f32, tag="ps2",
                                        name=f"ps2_{fb}_{h}_{ci}")
                           for ci in cig}
                    for k in range(MF):
                        lhsT = w2t[k][:, h * P:(h + 1) * P]
                        for ci in cig:
                            off, w = chunks[ci]
                            nc.tensor.matmul(psg[ci][:], lhsT,
                                             at[k][:, off:off + w],
                                             start=(k == 0), stop=(k == MF - 1))
                    for ci in cig:
                        off, w = chunks[ci]
                        if fb < NFB - 1:
                            nc.vector.tensor_add(y_acc[h][:, off:off + w],
                                                 y_acc[h][:, off:off + w],
                                                 psg[ci][:])
                        else:
                            # fused finalize: y = (psum + b2) + y_acc, then
                            # scale by the gate weight
                            nc.vector.scalar_tensor_tensor(
                                y_acc[h][:, off:off + w], psg[ci][:],
                                b2_sb[:, h:h + 1], y_acc[h][:, off:off + w],
                                OP.add, OP.add)
                            nc.vector.tensor_mul(y_acc[h][:, off:off + w],
                                                 y_acc[h][:, off:off + w],
                                                 comb_b[:, off:off + w])
                    if fb == NFB - 1 and (
                            (h % 2 == 1 and h < KH // 2) or h >= KH // 2):
                        # store finished rows while later h compute; the
                        # last four h-rows go singly so the final exposed
                        # flush is as small as possible
                        lo = h - 1 if h < KH // 2 else h
                        for ci in cig:
                            off, w = chunks[ci]
                            nc.sync.dma_start(
                                d_y[ci].ap()[:, lo:h + 1, :],
                                ybig[:, lo:h + 1, off:off + w])

    nc.compile()
    _BUILD_CACHE[C] = nc
    return nc


def _prepare(x, Wg, alpha, ln_w, ln_b, fc1_w, fc1_b, fc2_w, fc2_b):
    """Host-side routing, LayerNorm + per-core slab construction."""
    bfnp = ml_dtypes.bfloat16
    xf = np.asarray(x, np.float32).reshape(T, H)
    Wg = np.asarray(Wg, np.float32)
    alpha = np.asarray(alpha, np.float32)
    ln_w = np.asarray(ln_w, np.float32)
    ln_b = np.asarray(ln_b, np.float32)
    fc1_w = np.asarray(fc1_w, np.float32)
    fc1_b = np.asarray(fc1_b, np.float32)
    fc2_w = np.asarray(fc2_w, np.float32)
    fc2_b = np.asarray(fc2_b, np.float32)

    logits = xf @ Wg
    order = np.argsort(-logits, axis=1, kind="stable")
    top2 = order[:, :TOP_K]
    tv = np.take_along_axis(logits, top2, 1)
    sm = np.exp(tv - tv.max(1, keepdims=True))
    sm /= sm.sum(1, keepdims=True)
    comb = np.zeros((T, E), np.float32)
    np.put_along_axis(comb, top2, sm, 1)
    comb *= alpha
    sel = np.zeros((T, E), dtype=bool)
    sel[np.arange(T)[:, None], top2] = True
    idx = [np.nonzero(sel[:, e])[0] for e in range(E)]

    # apply LayerNorm on host (exact fp32); lnw/lnb are folded into W1/b1
    mu_t = xf.mean(1, keepdims=True)
    inv_t = 1.0 / np.sqrt(((xf - mu_t) ** 2).mean(1, keepdims=True) + LN_EPS)
    xn = (xf - mu_t) * inv_t

    maxc = max(len(i) for i in idx)
    C = max(512, 4 * ((maxc + 3) // 4))
    chunks = _chunks(C)

    in_maps = []
    for e in range(E):
        n = len(idx[e])
        xg = np.zeros((C, H), bfnp)
        xg[:n] = xn[idx[e]].astype(bfnp)
        xr = np.ascontiguousarray(xg.T).reshape(KH, P, C)
        cv = np.zeros(C, bfnp)
        cv[:n] = comb[idx[e], e].astype(bfnp)
        # LayerNorm scale/bias folded into fc1 (see module docstring)
        w1e = ln_w[e][:, None] * fc1_w[e]
        b1p = fc1_b[e] + ln_b[e] @ fc1_w[e]
        w1r = w1e.astype(bfnp).reshape(KH, P, F)
        w2r = fc2_w[e].astype(bfnp).reshape(F // P, P, H)
        im = {
            "b1r": np.ascontiguousarray(b1p.reshape(F // P, P).T),
            "b2": np.ascontiguousarray(fc2_b[e].reshape(KH, P).T),
            "combr": np.ascontiguousarray(
                np.broadcast_to(cv[None, :], (P, C))),
        }
        for ci, (off, w) in enumerate(chunks):
            im[f"x{ci}"] = np.ascontiguousarray(
                xr[:, :, off:off + w].transpose(1, 0, 2))
        WQ = FB // NW1
        for q in range(NW1):
            im[f"w1q{q}"] = np.ascontiguousarray(
                w1r[:, :, q * WQ:(q + 1) * WQ].transpose(1, 0, 2))
        HQ = H // NQ
        for q in range(NQ):
            im[f"w2q{q}"] = np.ascontiguousarray(
                w2r[0:MF, :, q * HQ:(q + 1) * HQ].transpose(1, 0, 2))
        for fb in range(1, NFB):
            im[f"w1b{fb}"] = np.ascontiguousarray(
                w1r[:, :, fb * FB:(fb + 1) * FB].transpose(1, 0, 2))
            im[f"w2b{fb}"] = np.ascontiguousarray(
                w2r[fb * MF:(fb + 1) * MF].transpose(1, 0, 2))
        in_maps.append(im)
    return in_maps, idx, C


def _kernel_impl(inputs, trace=False, trace_cores=None):
    from concourse import bass_utils

    in_maps, idx, C = _prepare(**inputs)
    chunks = _chunks(C)
    nc = _build(C)
    res = bass_utils.run_bass_kernel_spmd(
        nc, in_maps, core_ids=list(range(E)),
        trace=trace, trace_cores=trace_cores)

    out = np.zeros((T, H), np.float32)
    for e in range(E):
        yt = np.empty((H, C), np.float32)
        for ci, (off, w) in enumerate(chunks):
            slab = np.asarray(res.results[e][f"y{ci}"], np.float32)
            yt[:, off:off + w] = slab.transpose(1, 0, 2).reshape(H, w)
        n = len(idx[e])
        out[idx[e]] += yt.T[:n]
    return out.reshape(B, S, H), res


def kernel(**inputs):
    out, _ = _kernel_impl(inputs)
    return out


# revision 6
# speedup vs baseline: 1.1480x; 1.1480x over previous
"""Trainium2 Bass kernel for a top-2 gated MoE layer (8 experts, H=1024, F=4096).

Strategy (expert parallelism across the 8 NeuronCores):
  - Host computes the top-2 routing, the combine weights
    comb = softmax(top2) * alpha, and applies LayerNorm to x in fp32;
    it gathers each expert's tokens into a padded capacity-C block and
    pre-arranges EVERY device transfer as its own contiguous DRAM slab,
    already in SBUF [partition, k, col] layout.
  - Each core runs one expert (matmuls in bf16, fp32 PSUM accumulation).
  - Chunk widths are all >= ~233 columns so each LDWEIGHTS (~97 ns) hides
    under its matmuls; the first chunk is small (256) so compute starts
    as early as the DMA stream allows.
  - Startup-critical transfers (x chunk 0, w1 pieces) go first on both
    hardware DGE rings; small latency-bound tensors (b1, b2, comb) are
    single-packet transfers placed off the critical path.
  - Host scatter-adds the per-expert outputs back into the full [B,S,H].

Self-contained: shapes are hardcoded from the problem spec.
"""

import numpy as np
import ml_dtypes
from contextlib import ExitStack

TOP_K = 2
LN_EPS = 1e-5
B, S, H, E, F = 2, 2048, 1024, 8, 4096
T = B * S
P = 128
KH = H // P          # 8 H-tiles
FB = 1024            # F block size
NFB = F // FB        # 4 blocks
MF = FB // P         # 8 F-tiles per block
NQ = 4               # block-0 w2 quarters
NW1 = 8              # block-0 w1 pieces (one per F-tile)

_BUILD_CACHE = {}


def _chunks(C):
    # First chunk small (256) for an early compute start; middle chunks 512;
    # tail split so every chunk is in [234, 512] (LDWEIGHTS stays hidden).
    out = [(0, min(256, C))]
    off = out[0][1]
    rem = C - off
    while rem > 1024:
        out.append((off, 512))
        off += 512
        rem -= 512
    if rem > 512:
        a = min(512, rem - 234)
        out.append((off, a))
        off += a
        rem -= a
    if rem:
        out.append((off, rem))
    return out


def _build(C):
    """Build + compile the single-core Bass program (SPMD across 8 cores)."""
    if C in _BUILD_CACHE:
        return _BUILD_CACHE[C]

    import concourse.bass as bass  # noqa: F401
    import concourse.tile as tile
    import concourse.mybir as mybir
    from concourse import bacc

    bf = mybir.dt.bfloat16
    f32 = mybir.dt.float32
    AF = mybir.ActivationFunctionType
    OP = mybir.AluOpType

    nc = bacc.Bacc("TRN2", target_bir_lowering=False, debug=False, num_devices=8)

    chunks = _chunks(C)
    NC = len(chunks)
    d_x = [nc.dram_tensor(f"x{ci}", [P, KH, w], bf, kind="ExternalInput")
           for ci, (off, w) in enumerate(chunks)]
    d_w1q = [nc.dram_tensor(f"w1q{q}", [P, KH, FB // NW1], bf,
                            kind="ExternalInput") for q in range(NW1)]
    d_w2q = [nc.dram_tensor(f"w2q{q}", [P, MF, H // NQ], bf,
                            kind="ExternalInput") for q in range(NQ)]
    d_w1b = [nc.dram_tensor(f"w1b{fb}", [P, KH, FB], bf,
                            kind="ExternalInput") for fb in range(1, NFB)]
    d_w2b = [nc.dram_tensor(f"w2b{fb}", [P, MF, H], bf,
                            kind="ExternalInput") for fb in range(1, NFB)]
    d_b1r = nc.dram_tensor("b1r", [P, F // P], f32, kind="ExternalInput")
    d_b2 = nc.dram_tensor("b2", [P, KH], f32, kind="ExternalInput")
    d_comb = nc.dram_tensor("combr", [P, C], bf, kind="ExternalInput")
    d_y = [nc.dram_tensor(f"y{ci}", [P, KH, w], bf, kind="ExternalOutput")
           for ci, (off, w) in enumerate(chunks)]

    with tile.TileContext(nc) as tc, ExitStack() as ctx:
        const = ctx.enter_context(tc.tile_pool(name="const", bufs=1))
        bpool = ctx.enter_context(tc.tile_pool(name="bcast", bufs=1))
        xpool = ctx.enter_context(tc.tile_pool(name="x", bufs=1))
        w1a = ctx.enter_context(tc.tile_pool(name="w1a", bufs=NW1))
        w2a = ctx.enter_context(tc.tile_pool(name="w2a", bufs=NQ))
        w1pool = ctx.enter_context(tc.tile_pool(name="w1", bufs=2))
        w2pool = ctx.enter_context(tc.tile_pool(name="w2", bufs=2))
        apool = ctx.enter_context(tc.tile_pool(name="acts", bufs=8))
        ypool = ctx.enter_context(tc.tile_pool(name="yacc", bufs=1))
        ps1 = ctx.enter_context(tc.tile_pool(name="ps1", bufs=4, space="PSUM"))
        ps2 = ctx.enter_context(tc.tile_pool(name="ps2", bufs=4, space="PSUM"))

        # ---- PE warm-up: junk matmuls train the HAM clock gate toward
        # 2.4 GHz and bridge until the first x/w DMAs land ----
        ones_k = const.tile([P, 1], bf)
        nc.vector.memset(ones_k, 1.0)
        warm_rhs = const.tile([P, 512], bf)
        nc.vector.memset(warm_rhs, 0.0)
        ps_w = ps1.tile([1, 512], f32, tag="ps1", name="warm")
        for i in range(12):
            nc.tensor.matmul(ps_w[:], ones_k[:], warm_rhs[:],
                             start=True, stop=True)
        for i in range(4):
            nc.tensor.matmul(ps_w[:, 0:128], ones_k[:], warm_rhs[:, 0:128],
                             start=True, stop=True)

        xt = [xpool.tile([P, KH, w], bf, tag=f"xc{ci}", name=f"x_{ci}")
              for ci, (off, w) in enumerate(chunks)]
        w1q = [w1a.tile([P, KH, FB // NW1], bf, tag="w1a", name=f"w1a_{q}")
               for q in range(NW1)]
        w2q = [w2a.tile([P, MF, H // NQ], bf, tag="w2a", name=f"w2a_{q}")
               for q in range(NQ)]
        b1_sb = const.tile([P, F // P], f32)
        b2_sb = const.tile([P, KH], f32)
        comb_b = bpool.tile([P, C], bf)

        def xk(ci, k):
            return xt[ci][:, k, :]

        # fb0 processes chunk 0 first (smallest x, earliest start), then the
        # remaining chunks by ascending width — x DMA lands in that order
        c_order = [0] + sorted(range(1, NC), key=lambda ci: chunks[ci][1])

        # ---- DMA: startup-critical slabs first on BOTH hwdge rings, in
        # compute need-order; every x chunk split half/half across rings.
        # Small finalize tensors (b1/b2/comb) ride the gpsimd SW queue. ----
        for ci in c_order:
            nc.sync.dma_start(xt[ci][:, 0:KH // 2, :],
                              d_x[ci].ap()[:, 0:KH // 2, :])
            nc.scalar.dma_start(xt[ci][:, KH // 2:KH, :],
                                d_x[ci].ap()[:, KH // 2:KH, :])
            if ci == 0:
                for q in range(NW1):
                    eng = nc.sync if q % 2 == 0 else nc.scalar
                    eng.dma_start(w1q[q][:], d_w1q[q].ap())
        for q in range(NQ):
            eng = nc.sync if q % 2 == 0 else nc.scalar
            eng.dma_start(w2q[q][:], d_w2q[q].ap())
        nc.gpsimd.dma_start(b1_sb[:], d_b1r.ap())
        nc.gpsimd.dma_start(b2_sb[:], d_b2.ap())
        nc.gpsimd.dma_start(comb_b[:], d_comb.ap())

        ybig = ypool.tile([P, KH, C], bf, tag="yacc", name="ybig")
        y_acc = [ybig[:, h, :] for h in range(KH)]

        def load_w_block(fb):
            w1blk = w1pool.tile([P, KH, FB], bf, tag="w1", name=f"w1_{fb}")
            nc.sync.dma_start(w1blk[:], d_w1b[fb - 1].ap())
            w2blk = w2pool.tile([P, MF, H], bf, tag="w2", name=f"w2_{fb}")
            nc.scalar.dma_start(w2blk[:], d_w2b[fb - 1].ap())
            return ([w1blk[:, k, :] for k in range(KH)],
                    [w2blk[:, k, :] for k in range(MF)])

        at0 = [apool.tile([P, C], bf, tag="acts", name=f"a_0_{m}")
               for m in range(MF)]

        def fc1_fixup(psum, dst, fcol):
            # x is fully LayerNormed on host; just bias + gelu
            nc.scalar.activation(dst, psum[:], AF.Gelu_apprx_tanh,
                                 bias=b1_sb[:, fcol:fcol + 1])

        # ---- F-block 0: fc1 chunk by chunk (DMA-latency bridge: each chunk
        # starts as soon as its x lands), then fc2 grouped over all chunks
        # (one LDWEIGHTS per (h,k), chunk-rotated PSUM banks) ----
        for ci in c_order:
            off, w = chunks[ci]
            sl = slice(off, off + w)
            for m in range(MF):
                pst = ps1.tile([P, w], f32, tag="ps1", name=f"ps1_0_{m}_{ci}")
                for k in range(KH):
                    nc.tensor.matmul(pst[:], w1q[m][:, k, :],
                                     xk(ci, k),
                                     start=(k == 0), stop=(k == KH - 1))
                fc1_fixup(pst, at0[m][:, sl], m)
        for h in range(KH):
            w2piece = w2q[h // 2]
            hsl = slice((h % 2) * P, (h % 2) * P + P)
            psg = {ci: ps2.tile([P, chunks[ci][1]], f32, tag="ps2",
                                name=f"ps2_0_{h}_{ci}")
                   for ci in range(NC)}
            for k in range(MF):
                for ci in range(NC):
                    off, w = chunks[ci]
                    nc.tensor.matmul(psg[ci][:], w2piece[:, k, hsl],
                                     at0[k][:, off:off + w],
                                     start=(k == 0), stop=(k == MF - 1))
            for ci in range(NC):
                off, w = chunks[ci]
                nc.scalar.activation(y_acc[h][:, off:off + w], psg[ci][:],
                                     AF.Identity, bias=0.0)

        # ---- remaining F blocks: weight-stationary (each lhsT feeds all
        # chunks); the last block splits off the final chunk alone so the
        # finalize tail is short. ----
        for fb in range(1, NFB):
            w1t, w2t = load_w_block(fb)

            at = [apool.tile([P, C], bf, tag="acts", name=f"a_{fb}_{m}")
                  for m in range(MF)]
            if fb == NFB - 1 and NC > 1:
                groups = [list(range(NC - 1)), [NC - 1]]
            else:
                groups = [list(range(NC))]

            for cig in groups:
                for m in range(MF):
                    psg = {ci: ps1.tile([P, chunks[ci][1]], f32, tag="ps1",
                                        name=f"ps1_{fb}_{m}_{ci}")
                           for ci in cig}
                    for k in range(KH):
                        lhsT = w1t[k][:, m * P:(m + 1) * P]
                        for ci in cig:
                            nc.tensor.matmul(psg[ci][:], lhsT, xk(ci, k),
                                             start=(k == 0), stop=(k == KH - 1))
                    fcol = fb * MF + m
                    for ci in cig:
                        off, w = chunks[ci]
                        fc1_fixup(psg[ci], at[m][:, off:off + w], fcol)
                for h in range(KH):
                    psg = {ci: ps2.tile([P, chunks[ci][1]], f32, tag="ps2",
                                        name=f"ps2_{fb}_{h}_{ci}")
                           for ci in cig}
                    for k in range(MF):
                        lhsT = w2t[k][:, h * P:(h + 1) * P]
                        for ci in cig:
                            off, w = chunks[ci]
                            nc.tensor.matmul(psg[ci][:], lhsT,
                                             at[k][:, off:off + w],
                                             start=(k == 0), stop=(k == MF - 1))
                    for ci in cig:
                        off, w = chunks[ci]
                        if fb < NFB - 1:
                            nc.vector.tensor_add(y_acc[h][:, off:off + w],
                                                 y_acc[h][:, off:off + w],
                                                 psg[ci][:])
                        else:
                            # fused finalize: y = (psum + b2) + y_acc, then
                            # scale by the gate weight
                            nc.vector.scalar_tensor_tensor(
                                y_acc[h][:, off:off + w], psg[ci][:],
                                b2_sb[:, h:h + 1], y_acc[h][:, off:off + w],
                                OP.add, OP.add)
                            nc.vector.tensor_mul(y_acc[h][:, off:off + w],
                                                 y_acc[h][:, off:off + w],
                                                 comb_b[:, off:off + w])
                    if fb == NFB - 1 and (
                            (h % 2 == 1 and h < KH // 2) or h >= KH // 2):
                        # store finished rows while later h compute; the
                        # last four h-rows go singly so the final exposed
                        # flush is as small as possible
                        lo = h - 1 if h < KH // 2 else h
                        for ci in cig:
                            off, w = chunks[ci]
                            nc.sync.dma_start(
                                d_y[ci].ap()[:, lo:h + 1, :],
                                ybig[:, lo:h + 1, off:off + w])

    nc.compile()
    _BUILD_CACHE[C] = nc
    return nc


def _prepare(x, Wg, alpha, ln_w, ln_b, fc1_w, fc1_b, fc2_w, fc2_b):
    """Host-side routing, LayerNorm + per-core slab construction."""
    bfnp = ml_dtypes.bfloat16
    xf = np.asarray(x, np.float32).reshape(T, H)
    Wg = np.asarray(Wg, np.float32)
    alpha = np.asarray(alpha, np.float32)
    ln_w = np.asarray(ln_w, np.float32)
    ln_b = np.asarray(ln_b, np.float32)
    fc1_w = np.asarray(fc1_w, np.float32)
    fc1_b = np.asarray(fc1_b, np.float32)
    fc2_w = np.asarray(fc2_w, np.float32)
    fc2_b = np.asarray(fc2_b, np.float32)

    logits = xf @ Wg
    order = np.argsort(-logits, axis=1, kind="stable")
    top2 = order[:, :TOP_K]
    tv = np.take_along_axis(logits, top2, 1)
    sm = np.exp(tv - tv.max(1, keepdims=True))
    sm /= sm.sum(1, keepdims=True)
    comb = np.zeros((T, E), np.float32)
    np.put_along_axis(comb, top2, sm, 1)
    comb *= alpha
    sel = np.zeros((T, E), dtype=bool)
    sel[np.arange(T)[:, None], top2] = True
    idx = [np.nonzero(sel[:, e])[0] for e in range(E)]

    # apply LayerNorm on host (exact fp32); lnw/lnb are folded into W1/b1
    mu_t = xf.mean(1, keepdims=True)
    inv_t = 1.0 / np.sqrt(((xf - mu_t) ** 2).mean(1, keepdims=True) + LN_EPS)
    xn = (xf - mu_t) * inv_t

    maxc = max(len(i) for i in idx)
    C = max(512, 4 * ((maxc + 3) // 4))
    chunks = _chunks(C)

    in_maps = []
    for e in range(E):
        n = len(idx[e])
        xg = np.zeros((C, H), bfnp)
        xg[:n] = xn[idx[e]].astype(bfnp)
        xr = np.ascontiguousarray(xg.T).reshape(KH, P, C)
        cv = np.zeros(C, bfnp)
        cv[:n] = comb[idx[e], e].astype(bfnp)
        # LayerNorm scale/bias folded into fc1 (see module docstring)
        w1e = ln_w[e][:, None] * fc1_w[e]
        b1p = fc1_b[e] + ln_b[e] @ fc1_w[e]
        w1r = w1e.astype(bfnp).reshape(KH, P, F)
        w2r = fc2_w[e].astype(bfnp).reshape(F // P, P, H)
        im = {
            "b1r": np.ascontiguousarray(b1p.reshape(F // P, P).T),
            "b2": np.ascontiguousarray(fc2_b[e].reshape(KH, P).T),
            "combr": np.ascontiguousarray(
                np.broadcast_to(cv[None, :], (P, C))),
        }
        for ci, (off, w) in enumerate(chunks):
            im[f"x{ci}"] = np.ascontiguousarray(
                xr[:, :, off:off + w].transpose(1, 0, 2))
        WQ = FB // NW1
        for q in range(NW1):
            im[f"w1q{q}"] = np.ascontiguousarray(
                w1r[:, :, q * WQ:(q + 1) * WQ].transpose(1, 0, 2))
        HQ = H // NQ
        for q in range(NQ):
            im[f"w2q{q}"] = np.ascontiguousarray(
                w2r[0:MF, :, q * HQ:(q + 1) * HQ].transpose(1, 0, 2))
        for fb in range(1, NFB):
            im[f"w1b{fb}"] = np.ascontiguousarray(
                w1r[:, :, fb * FB:(fb + 1) * FB].transpose(1, 0, 2))
            im[f"w2b{fb}"] = np.ascontiguousarray(
                w2r[fb * MF:(fb + 1) * MF].transpose(1, 0, 2))
        in_maps.append(im)
    return in_maps, idx, C


def _kernel_impl(inputs, trace=False, trace_cores=None):
    from concourse import bass_utils

    in_maps, idx, C = _prepare(**inputs)
    chunks = _chunks(C)
    nc = _build(C)
    res = bass_utils.run_bass_kernel_spmd(
        nc, in_maps, core_ids=list(range(E)),
        trace=trace, trace_cores=trace_cores)

    out = np.zeros((T, H), np.float32)
    for e in range(E):
        yt = np.empty((H, C), np.float32)
        for ci, (off, w) in enumerate(chunks):
            slab = np.asarray(res.results[e][f"y{ci}"], np.float32)
            yt[:, off:off + w] = slab.transpose(1, 0, 2).reshape(H, w)
        n = len(idx[e])
        out[idx[e]] += yt.T[:n]
    return out.reshape(B, S, H), res


def kernel(**inputs):
    out, _ = _kernel_impl(inputs)
    return out


# revision 7
# speedup vs baseline: 1.1894x; 1.0361x over previous
"""Trainium2 Bass kernel for a top-2 gated MoE layer (8 experts, H=1024, F=4096).

Strategy (expert parallelism across the 8 NeuronCores):
  - Host computes the top-2 routing, the combine weights
    comb = softmax(top2) * alpha, and applies LayerNorm to x in fp32;
    it gathers each expert's tokens into a padded capacity-C block and
    pre-arranges EVERY device transfer as its own contiguous DRAM slab,
    already in SBUF [partition, k, col] layout.
  - Each core runs one expert (matmuls in bf16, fp32 PSUM accumulation).
  - Chunk widths are all >= ~233 columns so each LDWEIGHTS (~97 ns) hides
    under its matmuls; the first chunk is small (256) so compute starts
    as early as the DMA stream allows.
  - Each hardware DGE ring executes ONE transfer at a time (~1 us fixed
    overhead + bytes/rate), so startup data is packed into few transfers
    issued in exact first-use order across the two rings; tiny finalize
    tensors (b1/b2/comb) ride the gpsimd software DMA queue.
  - fb0 fc1 runs chunk by chunk (DMA bridge); everything else interleaves
    all chunks per LDWEIGHTS so weight loads hide and PSUM banks rotate.
  - Host scatter-adds the per-expert outputs back into the full [B,S,H].

Self-contained: shapes are hardcoded from the problem spec.
"""

import numpy as np
import ml_dtypes
from contextlib import ExitStack

TOP_K = 2
LN_EPS = 1e-5
B, S, H, E, F = 2, 2048, 1024, 8, 4096
T = B * S
P = 128
KH = H // P          # 8 H-tiles
FB = 1024            # F block size
NFB = F // FB        # 4 blocks
MF = FB // P         # 8 F-tiles per block

# fb0 fc1 weight pieces (cols of W1 block 0): sized so the m-loop never
# outruns the DMA stream while transfer count stays low
W1PIECES = [128, 128, 256, 512]
# fb0 fc2 weight halves (cols of W2 block 0)
W2PIECES = [512, 512]

_BUILD_CACHE = {}


def _chunks(C):
    # First chunk small (256) for an early compute start; middle chunks 512;
    # tail split so every chunk is in [234, 512] (LDWEIGHTS stays hidden).
    out = [(0, min(256, C))]
    off = out[0][1]
    rem = C - off
    while rem > 1024:
        out.append((off, 512))
        off += 512
        rem -= 512
    if rem > 512:
        a = min(512, rem - 234)
        out.append((off, a))
        off += a
        rem -= a
    if rem:
        out.append((off, rem))
    return out


def _build(C):
    """Build + compile the single-core Bass program (SPMD across 8 cores)."""
    if C in _BUILD_CACHE:
        return _BUILD_CACHE[C]

    import concourse.bass as bass  # noqa: F401
    import concourse.tile as tile
    import concourse.mybir as mybir
    from concourse import bacc

    bf = mybir.dt.bfloat16
    f32 = mybir.dt.float32
    AF = mybir.ActivationFunctionType
    OP = mybir.AluOpType

    nc = bacc.Bacc("TRN2", target_bir_lowering=False, debug=False, num_devices=8)

    chunks = _chunks(C)
    NC = len(chunks)
    d_x = [nc.dram_tensor(f"x{ci}", [P, KH, w], bf, kind="ExternalInput")
           for ci, (off, w) in enumerate(chunks)]
    d_w1q = [nc.dram_tensor(f"w1q{q}", [P, KH, w], bf, kind="ExternalInput")
             for q, w in enumerate(W1PIECES)]
    d_w2q = [nc.dram_tensor(f"w2q{q}", [P, MF, w], bf, kind="ExternalInput")
             for q, w in enumerate(W2PIECES)]
    d_w1b = [nc.dram_tensor(f"w1b{fb}", [P, KH, FB], bf,
                            kind="ExternalInput") for fb in range(1, NFB)]
    d_w2b = [nc.dram_tensor(f"w2b{fb}", [P, MF, H], bf,
                            kind="ExternalInput") for fb in range(1, NFB)]
    d_b1r = nc.dram_tensor("b1r", [P, F // P], f32, kind="ExternalInput")
    d_b2 = nc.dram_tensor("b2", [P, KH], f32, kind="ExternalInput")
    d_comb = nc.dram_tensor("combr", [P, C], bf, kind="ExternalInput")
    d_y = [nc.dram_tensor(f"y{ci}", [P, KH, w], bf, kind="ExternalOutput")
           for ci, (off, w) in enumerate(chunks)]

    with tile.TileContext(nc) as tc, ExitStack() as ctx:
        const = ctx.enter_context(tc.tile_pool(name="const", bufs=1))
        bpool = ctx.enter_context(tc.tile_pool(name="bcast", bufs=1))
        xpool = ctx.enter_context(tc.tile_pool(name="x", bufs=1))
        w1a = ctx.enter_context(tc.tile_pool(name="w1a", bufs=len(W1PIECES)))
        w2a = ctx.enter_context(tc.tile_pool(name="w2a", bufs=len(W2PIECES)))
        w1pool = ctx.enter_context(tc.tile_pool(name="w1", bufs=2))
        w2pool = ctx.enter_context(tc.tile_pool(name="w2", bufs=2))
        apool = ctx.enter_context(tc.tile_pool(name="acts", bufs=8))
        ypool = ctx.enter_context(tc.tile_pool(name="yacc", bufs=1))
        ps1 = ctx.enter_context(tc.tile_pool(name="ps1", bufs=4, space="PSUM"))
        ps2 = ctx.enter_context(tc.tile_pool(name="ps2", bufs=4, space="PSUM"))

        # ---- PE warm-up: junk matmuls train the HAM clock gate toward
        # 2.4 GHz and bridge until the first x/w DMAs land (~12.3us) ----
        ones_k = const.tile([P, 1], bf)
        nc.vector.memset(ones_k, 1.0)
        warm_rhs = const.tile([P, 512], bf)
        nc.vector.memset(warm_rhs, 0.0)
        ps_w = ps1.tile([1, 512], f32, tag="ps1", name="warm")
        for i in range(15):
            nc.tensor.matmul(ps_w[:], ones_k[:], warm_rhs[:],
                             start=True, stop=True)

        xt = [xpool.tile([P, KH, w], bf, tag=f"xc{ci}", name=f"x_{ci}")
              for ci, (off, w) in enumerate(chunks)]
        w1q = [w1a.tile([P, KH, w], bf, tag="w1a", name=f"w1a_{q}")
               for q, w in enumerate(W1PIECES)]
        w2q = [w2a.tile([P, MF, w], bf, tag="w2a", name=f"w2a_{q}")
               for q, w in enumerate(W2PIECES)]
        b1_sb = const.tile([P, F // P], f32)
        b2_sb = const.tile([P, KH], f32)
        comb_b = bpool.tile([P, C], bf)

        def xk(ci, k):
            return xt[ci][:, k, :]

        def w1_piece(m):
            # (tile, column sub-slice) for F-tile m of block 0
            base = 0
            for q, w in enumerate(W1PIECES):
                if m * P < base + w:
                    s = m * P - base
                    return w1q[q][:, :, s:s + P]
                base += w
            raise AssertionError

        def w2_piece(h):
            base = 0
            for q, w in enumerate(W2PIECES):
                if h * P < base + w:
                    s = h * P - base
                    return w2q[q][:, :, s:s + P]
                base += w
            raise AssertionError

        # fb0 processes chunk 0 first (smallest x, earliest start), then the
        # remaining chunks by ascending width — x DMA lands in that order
        c_order = [0] + sorted(range(1, NC), key=lambda ci: chunks[ci][1])

        # ---- DMA: each ring runs one transfer at a time, so emit few,
        # need-ordered transfers. Scalar ring leads with the first weight
        # pieces (sync carries x0), then the rings split the rest. ----
        nc.sync.dma_start(xt[0][:], d_x[0].ap())
        nc.scalar.dma_start(w1q[0][:], d_w1q[0].ap())
        nc.scalar.dma_start(w1q[1][:], d_w1q[1].ap())
        nc.sync.dma_start(w1q[2][:], d_w1q[2].ap())
        nc.scalar.dma_start(w1q[3][:], d_w1q[3].ap())
        for ci in c_order[1:]:
            nc.sync.dma_start(xt[ci][:], d_x[ci].ap())
        nc.scalar.dma_start(w2q[1][:], d_w2q[1].ap())
        nc.sync.dma_start(w2q[0][:], d_w2q[0].ap())
        nc.gpsimd.dma_start(b1_sb[:], d_b1r.ap())
        nc.gpsimd.dma_start(b2_sb[:], d_b2.ap())
        nc.gpsimd.dma_start(comb_b[:], d_comb.ap())

        ybig = ypool.tile([P, KH, C], bf, tag="yacc", name="ybig")
        y_acc = [ybig[:, h, :] for h in range(KH)]

        def load_w_block(fb):
            w1blk = w1pool.tile([P, KH, FB], bf, tag="w1", name=f"w1_{fb}")
            nc.sync.dma_start(w1blk[:], d_w1b[fb - 1].ap())
            w2blk = w2pool.tile([P, MF, H], bf, tag="w2", name=f"w2_{fb}")
            nc.scalar.dma_start(w2blk[:], d_w2b[fb - 1].ap())
            return ([w1blk[:, k, :] for k in range(KH)],
                    [w2blk[:, k, :] for k in range(MF)])

        at0 = [apool.tile([P, C], bf, tag="acts", name=f"a_0_{m}")
               for m in range(MF)]

        def fc1_fixup(psum, dst, fcol):
            # x is fully LayerNormed on host; just bias + gelu
            nc.scalar.activation(dst, psum[:], AF.Gelu_apprx_tanh,
                                 bias=b1_sb[:, fcol:fcol + 1])

        # ---- F-block 0: fc1 chunk by chunk (DMA-latency bridge: each chunk
        # starts as soon as its x lands), then fc2 grouped over all chunks
        # (one LDWEIGHTS per (h,k), chunk-rotated PSUM banks) ----
        for ci in c_order:
            off, w = chunks[ci]
            sl = slice(off, off + w)
            for m in range(MF):
                pst = ps1.tile([P, w], f32, tag="ps1", name=f"ps1_0_{m}_{ci}")
                piece = w1_piece(m)
                for k in range(KH):
                    nc.tensor.matmul(pst[:], piece[:, k, :],
                                     xk(ci, k),
                                     start=(k == 0), stop=(k == KH - 1))
                fc1_fixup(pst, at0[m][:, sl], m)
        for h in range(KH):
            piece = w2_piece(h)
            psg = {ci: ps2.tile([P, chunks[ci][1]], f32, tag="ps2",
                                name=f"ps2_0_{h}_{ci}")
                   for ci in range(NC)}
            for k in range(MF):
                for ci in range(NC):
                    off, w = chunks[ci]
                    nc.tensor.matmul(psg[ci][:], piece[:, k, :],
                                     at0[k][:, off:off + w],
                                     start=(k == 0), stop=(k == MF - 1))
            for ci in range(NC):
                off, w = chunks[ci]
                nc.scalar.activation(y_acc[h][:, off:off + w], psg[ci][:],
                                     AF.Identity, bias=0.0)

        # ---- remaining F blocks: weight-stationary (each lhsT feeds all
        # chunks); the last block splits off the final chunk alone so the
        # finalize tail is short. ----
        for fb in range(1, NFB):
            w1t, w2t = load_w_block(fb)

            at = [apool.tile([P, C], bf, tag="acts", name=f"a_{fb}_{m}")
                  for m in range(MF)]
            if fb == NFB - 1 and NC > 1:
                groups = [list(range(NC - 1)), [NC - 1]]
            else:
                groups = [list(range(NC))]

            for cig in groups:
                for m in range(MF):
                    psg = {ci: ps1.tile([P, chunks[ci][1]], f32, tag="ps1",
                                        name=f"ps1_{fb}_{m}_{ci}")
                           for ci in cig}
                    for k in range(KH):
                        lhsT = w1t[k][:, m * P:(m + 1) * P]
                        for ci in cig:
                            nc.tensor.matmul(psg[ci][:], lhsT, xk(ci, k),
                                             start=(k == 0), stop=(k == KH - 1))
                    fcol = fb * MF + m
                    for ci in cig:
                        off, w = chunks[ci]
                        fc1_fixup(psg[ci], at[m][:, off:off + w], fcol)
                for h in range(KH):
                    psg = {ci: ps2.tile([P, chunks[ci][1]], f32, tag="ps2",
                                        name=f"ps2_{fb}_{h}_{ci}")
                           for ci in cig}
                    for k in range(MF):
                        lhsT = w2t[k][:, h * P:(h + 1) * P]
                        for ci in cig:
                            off, w = chunks[ci]
                            nc.tensor.matmul(psg[ci][:], lhsT,
                                             at[k][:, off:off + w],
                                             start=(k == 0), stop=(k == MF - 1))
                    for ci in cig:
                        off, w = chunks[ci]
                        if fb < NFB - 1:
                            nc.vector.tensor_add(y_acc[h][:, off:off + w],
                                                 y_acc[h][:, off:off + w],
                                                 psg[ci][:])
                        else:
                            # fused finalize: y = (psum + b2) + y_acc, then
                            # scale by the gate weight
                            nc.vector.scalar_tensor_tensor(
                                y_acc[h][:, off:off + w], psg[ci][:],
                                b2_sb[:, h:h + 1], y_acc[h][:, off:off + w],
                                OP.add, OP.add)
                            nc.vector.tensor_mul(y_acc[h][:, off:off + w],
                                                 y_acc[h][:, off:off + w],
                                                 comb_b[:, off:off + w])
                    if fb == NFB - 1 and cig[-1] == NC - 1:
                        # last group: store progressively so the final
                        # exposed flush is at most two rows
                        if h == KH // 2 - 1 or h == KH - 3 or h == KH - 1:
                            lo = 0 if h == KH // 2 - 1 else h - 1
                            for ci in cig:
                                off, w = chunks[ci]
                                nc.sync.dma_start(
                                    d_y[ci].ap()[:, lo:h + 1, :],
                                    ybig[:, lo:h + 1, off:off + w])
                if fb == NFB - 1 and cig[-1] != NC - 1:
                    # non-final groups: one whole-chunk store each, fired
                    # while the last group computes
                    for ci in cig:
                        off, w = chunks[ci]
                        nc.sync.dma_start(d_y[ci].ap()[:],
                                          ybig[:, :, off:off + w])

    nc.compile()
    _BUILD_CACHE[C] = nc
    return nc


def _prepare(x, Wg, alpha, ln_w, ln_b, fc1_w, fc1_b, fc2_w, fc2_b):
    """Host-side routing, LayerNorm + per-core slab construction."""
    bfnp = ml_dtypes.bfloat16
    xf = np.asarray(x, np.float32).reshape(T, H)
    Wg = np.asarray(Wg, np.float32)
    alpha = np.asarray(alpha, np.float32)
    ln_w = np.asarray(ln_w, np.float32)
    ln_b = np.asarray(ln_b, np.float32)
    fc1_w = np.asarray(fc1_w, np.float32)
    fc1_b = np.asarray(fc1_b, np.float32)
    fc2_w = np.asarray(fc2_w, np.float32)
    fc2_b = np.asarray(fc2_b, np.float32)

    logits = xf @ Wg
    order = np.argsort(-logits, axis=1, kind="stable")
    top2 = order[:, :TOP_K]
    tv = np.take_along_axis(logits, top2, 1)
    sm = np.exp(tv - tv.max(1, keepdims=True))
    sm /= sm.sum(1, keepdims=True)
    comb = np.zeros((T, E), np.float32)
    np.put_along_axis(comb, top2, sm, 1)
    comb *= alpha
    sel = np.zeros((T, E), dtype=bool)
    sel[np.arange(T)[:, None], top2] = True
    idx = [np.nonzero(sel[:, e])[0] for e in range(E)]

    # apply LayerNorm on host (exact fp32); lnw/lnb are folded into W1/b1
    mu_t = xf.mean(1, keepdims=True)
    inv_t = 1.0 / np.sqrt(((xf - mu_t) ** 2).mean(1, keepdims=True) + LN_EPS)
    xn = (xf - mu_t) * inv_t

    maxc = max(len(i) for i in idx)
    C = max(512, 4 * ((maxc + 3) // 4))
    chunks = _chunks(C)

    in_maps = []
    for e in range(E):
        n = len(idx[e])
        xg = np.zeros((C, H), bfnp)
        xg[:n] = xn[idx[e]].astype(bfnp)
        xr = np.ascontiguousarray(xg.T).reshape(KH, P, C)
        cv = np.zeros(C, bfnp)
        cv[:n] = comb[idx[e], e].astype(bfnp)
        # LayerNorm scale/bias folded into fc1 (see module docstring)
        w1e = ln_w[e][:, None] * fc1_w[e]
        b1p = fc1_b[e] + ln_b[e] @ fc1_w[e]
        w1r = w1e.astype(bfnp).reshape(KH, P, F)
        w2r = fc2_w[e].astype(bfnp).reshape(F // P, P, H)
        im = {
            "b1r": np.ascontiguousarray(b1p.reshape(F // P, P).T),
            "b2": np.ascontiguousarray(fc2_b[e].reshape(KH, P).T),
            "combr": np.ascontiguousarray(
                np.broadcast_to(cv[None, :], (P, C))),
        }
        for ci, (off, w) in enumerate(chunks):
            im[f"x{ci}"] = np.ascontiguousarray(
                xr[:, :, off:off + w].transpose(1, 0, 2))
        base = 0
        for q, w in enumerate(W1PIECES):
            im[f"w1q{q}"] = np.ascontiguousarray(
                w1r[:, :, base:base + w].transpose(1, 0, 2))
            base += w
        base = 0
        for q, w in enumerate(W2PIECES):
            im[f"w2q{q}"] = np.ascontiguousarray(
                w2r[0:MF, :, base:base + w].transpose(1, 0, 2))
            base += w
        for fb in range(1, NFB):
            im[f"w1b{fb}"] = np.ascontiguousarray(
                w1r[:, :, fb * FB:(fb + 1) * FB].transpose(1, 0, 2))
            im[f"w2b{fb}"] = np.ascontiguousarray(
                w2r[fb * MF:(fb + 1) * MF].transpose(1, 0, 2))
        in_maps.append(im)
    return in_maps, idx, C


def _kernel_impl(inputs, trace=False, trace_cores=None):
    from concourse import bass_utils

    in_maps, idx, C = _prepare(**inputs)
    chunks = _chunks(C)
    nc = _build(C)
    res = bass_utils.run_bass_kernel_spmd(
        nc, in_maps, core_ids=list(range(E)),
        trace=trace, trace_cores=trace_cores)

    out = np.zeros((T, H), np.float32)
    for e in range(E):
        yt = np.empty((H, C), np.float32)
        for ci, (off, w) in enumerate(chunks):
            slab = np.asarray(res.results[e][f"y{ci}"], np.float32)
            yt[:, off:off + w] = slab.transpose(1, 0, 2).reshape(H, w)
        n = len(idx[e])
        out[idx[e]] += yt.T[:n]
    return out.reshape(B, S, H), res


def kernel(**inputs):
    out, _ = _kernel_impl(inputs)
    return out
